# revision 11
# baseline (speedup 1.0000x reference)
"""Trainium2 Bass kernel: multi-head attention with per-head QK LayerNorm.

Problem shapes: B=2, S=2048, D=1024, H=16 heads, head_dim=64, fp32 I/O.

Sharding (8 cores): core c handles batch b = c//4 and head-group g = c%4
(4 heads = 256 qkv dims). Each core computes its heads' attention and a
partial out-projection; the host sums the 4 partials per batch entry
(tensor-parallel all-reduce done on host at unshard time) and adds o_b.

Algebraic restructurings (same as the f32r baseline, all exact modulo
fp rounding):
  - LN mean subtraction and gain g are linear => folded into q_w/k_w
    (and biases) on the host.
  - LN variance = sum(w_d * qg_d^2) with w_d = 1/(64*g_d^2): computed on
    device from qg^2 via small matmuls with block-diagonal weights.
  - rstd_q folded into qT columns and tau*rstd_k into kT columns (via
    partition-broadcast DMAs), so softmax is a bare exp() of raw scores.
  - max-subtraction skipped: post-LN rows have norm 8 => |scores| <= 8.
  - sum(exp) over kv falls out of the AV matmul via a ones-column
    appended to V; normalization happens on attT eviction.

v2 changes (from NTFF profile analysis of the f32r baseline at 605us):
  - ALL matmul operands are bf16 (PSUM accumulation stays fp32).  On HW
    the f32r path ran ~2-4x slower than modeled: fp32 weights disable
    fast-weight-load and the PE sat at the HAM-throttled 1.2 GHz clock
    nearly the whole run (throttle_active 516us at util limit 0.5).
    bf16 runs 1 cycle/row warm with FWL, and halves SBUF/DMA traffic.
  - Projection matmuls use N=512 moving blocks (bf16 allows up to 1024).
  - LN stats matmuls are deferred one block behind their projections so
    the strict-FIFO PE queue never waits on the ACT square.
  - exp() reads scores straight from PSUM in [128, 1024] groups (a DVE
    PSUM->SBUF eviction measured ~1 elem/cycle/lane and became the
    bottleneck when tried), writing bf16 probs to SBUF for AV.
  - AV matmuls are software-pipelined one (q-block, head) unit behind
    QK, emitted MID-unit so the strict-FIFO PE queue has work while the
    exp drains this unit's score PSUM banks.
  - x is DMA'd in (k-chunk, s-block) slices ordered s-block-major so
    the first projection group starts after ~1MB, not the whole 4MB.
  - Partition-broadcast DMAs (LN rstd, softmax denom) ride the idle
    GpSimd DGE queue: on the Sync FIFO they sat behind the bulk input
    loads and stalled the DVE->PE stats chain ~15us per block.
"""

import os
import sys

import numpy as np

for _p in ("/opt/trn_rl_repo",):
    if _p not in sys.path:
        sys.path.append(_p)

# ---- problem constants (hardcoded; kernel.py must be self-contained) ----
B, S, D, H, HD = 2, 2048, 1024, 16, 64
EPS = 1e-5
NCORES = 8
GPC = 4            # cores per batch entry (head-groups)
HL = H // GPC      # 4 local heads
DL = HL * HD       # 256 local qkv dims
P = 128
KC = D // P        # 8 contraction chunks for projections
CL = DL // P       # 2 local-dim partition chunks
SB = 512           # projection free-dim block
NSB = S // SB      # 4 blocks
NKV = S // P       # 16 kv chunks
QB = 256           # attention q-block
NQB = S // QB      # 8 q-blocks

_CACHE = {}


class _Thunk:
    __slots__ = ("emit", "av")

    def __init__(self, emit, av):
        self.emit = emit
        self.av = av


def _build_nc():
    """Build the (single, SPMD-shared) Bass program for one core."""
    import concourse.bass as bass
    import concourse.mybir as mybir
    import concourse.tile as tile
    from concourse import bacc
    from concourse.dve_ops import RECIPROCAL_APPROX_FAST, RECIP_APPROX_FAST_CONSTS

    f32 = mybir.dt.float32
    bf16 = mybir.dt.bfloat16
    AF = mybir.ActivationFunctionType
    rc = RECIP_APPROX_FAST_CONSTS

    def recip(nc, out, in_):
        # ~51-ULP reciprocal in a single DVE pass (vs ~6 cyc/elem exact).
        return nc.vector._custom_dve(
            RECIPROCAL_APPROX_FAST, out=out, in0=in_,
            s0=rc["s0"], s1=rc["s1"], imm2=rc["imm2"],
        )

    nc = bacc.Bacc(trn_type="TRN2")

    xT_d = nc.dram_tensor("xT", [KC, P, S], bf16, kind="ExternalInput")
    wqT_d = nc.dram_tensor("wqT", [KC, P, DL], bf16, kind="ExternalInput")
    wkT_d = nc.dram_tensor("wkT", [KC, P, DL], bf16, kind="ExternalInput")
    wvT_d = nc.dram_tensor("wvT", [KC, P, DL], bf16, kind="ExternalInput")
    woT_d = nc.dram_tensor("woT", [CL, P, D], bf16, kind="ExternalInput")
    qb_d = nc.dram_tensor("qb", [CL, P, 1], f32, kind="ExternalInput")
    kb_d = nc.dram_tensor("kb", [CL, P, 1], f32, kind="ExternalInput")
    vb_d = nc.dram_tensor("vb", [1, DL], f32, kind="ExternalInput")
    wsq_d = nc.dram_tensor("wsq", [CL, P, HL], bf16, kind="ExternalInput")
    wsk_d = nc.dram_tensor("wsk", [CL, P, HL], bf16, kind="ExternalInput")
    out_d = nc.dram_tensor("out", [NKV, P, D], f32, kind="ExternalOutput")

    with tile.TileContext(nc) as tc:
        with tc.tile_pool(name="big", bufs=1) as big:
            # ---- persistent SBUF; q/k weights first, then x s-block-major
            # so the first projection group is runnable ASAP ----
            wq_sb, wk_sb, wv_sb = [], [], []
            for wlist, wd, nm in ((wk_sb, wkT_d, "wk"), (wq_sb, wqT_d, "wq")):
                for k in range(KC):
                    t = big.tile([P, DL], bf16, name=f"{nm}{k}")
                    nc.sync.dma_start(t, wd[k])
                    wlist.append(t)
            xt = [big.tile([P, S], bf16, name=f"xt{k}") for k in range(KC)]
            for sb in range(NSB):
                for k in range(KC):
                    nc.sync.dma_start(
                        xt[k][:, sb * SB:(sb + 1) * SB],
                        xT_d[k][:, sb * SB:(sb + 1) * SB])
            for k in range(KC):
                t = big.tile([P, DL], bf16, name=f"wv{k}")
                nc.sync.dma_start(t, wvT_d[k])
                wv_sb.append(t)
            wo_sb = big.tile([P, CL, D], bf16, name="wo_sb")
            for c in range(CL):
                nc.sync.dma_start(wo_sb[:, c, :], woT_d[c])
            qb_sb = big.tile([P, CL, 1], f32, name="qb_sb")
            kb_sb = big.tile([P, CL, 1], f32, name="kb_sb")
            for c in range(CL):
                nc.sync.dma_start(qb_sb[:, c, :], qb_d[c])
                nc.sync.dma_start(kb_sb[:, c, :], kb_d[c])
            vb_bc = big.tile([P, DL], f32, name="vb_bc")
            nc.sync.dma_start(vb_bc, vb_d[:].to_broadcast((P, DL)))
            wsq_sb = big.tile([P, CL, HL], bf16, name="wsq_sb")
            wsk_sb = big.tile([P, CL, HL], bf16, name="wsk_sb")
            for c in range(CL):
                nc.sync.dma_start(wsq_sb[:, c, :], wsq_d[c])
                nc.sync.dma_start(wsk_sb[:, c, :], wsk_d[c])

            kT_sb = big.tile([P, CL, S], bf16, name="kT_sb")
            qTs_sb = big.tile([P, CL, S], bf16, name="qTs_sb")
            vaug_sb = big.tile([P, NKV, HL, HD + 1], bf16, name="vaug_sb")
            attT_sb = big.tile([P, CL, S], bf16, name="attT_sb")
            nc.vector.memset(vaug_sb[:, :, :, HD:HD + 1], 1.0)
            eps_q = big.tile([P, 1], f32, name="eps_q")
            nc.vector.memset(eps_q, EPS)
            eps_k = big.tile([P, 1], f32, name="eps_k")
            nc.vector.memset(eps_k, 64.0 * EPS)

            # ============ phase 1: projections + LN stat folding ===========
            # For q and k: project (PE), evict+bias to bf16 SBUF (DVE),
            # square (ACT), variance via block-diagonal stats matmul (PE,
            # DEFERRED one (name,sb) block so the PE FIFO never stalls on
            # the ACT square), sqrt+eps (ACT), reciprocal (DVE), partition-
            # broadcast the per-(head,s) scale (DMA), multiply into qTs/kT
            # (DVE).  tau=1/8 is folded into the k-side via sqrt scale=64.
            with tc.tile_pool(name="pj", bufs=3, space="PSUM") as pj, \
                 tc.tile_pool(name="st", bufs=2, space="PSUM") as st, \
                 tc.tile_pool(name="sq", bufs=6) as sq, \
                 tc.tile_pool(name="ev", bufs=3) as ev:

                deferred = []

                def run_deferred():
                    while deferred:
                        deferred.pop(0)()

                def make_stats(name, sb, trs, qsqs, wst, dst, eps_t, sc):
                    def emit():
                        stp = st.tile([HL, SB], f32, name="st_t")
                        for c in range(CL):
                            nc.tensor.matmul(
                                stp, wst[:, c, :], qsqs[c],
                                start=(c == 0), stop=(c == CL - 1),
                            )
                        stmp = ev.tile([HL, SB], f32, name="stmp")
                        nc.scalar.activation(stmp, stp, AF.Sqrt,
                                             bias=eps_t[:HL], scale=sc)
                        rr = ev.tile([HL, SB], f32, name="rr")
                        recip(nc, rr, stmp)
                        for c in range(CL):
                            qsc = ev.tile([P, SB], f32, name="qsc")
                            # gpsimd DGE queue: keeps this latency-critical
                            # broadcast off the Sync FIFO (which is busy
                            # streaming the bulk x/w loads).
                            nc.gpsimd.dma_start(
                                qsc,
                                rr[c * 2:(c + 1) * 2, None, :]
                                .to_broadcast((2, HD, SB)),
                            )
                            nc.vector.tensor_mul(
                                dst[:, c, sb * SB:(sb + 1) * SB], trs[c], qsc)
                    return emit

                for name, wlist, bcol, wst, dst, eps_t, sc in (
                        ("k", wk_sb, kb_sb, wsk_sb, kT_sb, eps_k, 64.0),
                        ("q", wq_sb, qb_sb, wsq_sb, qTs_sb, eps_q, 1.0)):
                    for sb in range(NSB):
                        trs, qsqs = [], []
                        for c in range(CL):
                            ph = pj.tile([P, SB], f32, name="pj_t")
                            for k in range(KC):
                                nc.tensor.matmul(
                                    ph, wlist[k][:, c * P:(c + 1) * P],
                                    xt[k][:, sb * SB:(sb + 1) * SB],
                                    start=(k == 0), stop=(k == KC - 1),
                                )
                            tr = sq.tile([P, SB], bf16, name="tr_t")
                            nc.vector.tensor_scalar_add(tr, ph, bcol[:, c, :])
                            trs.append(tr)
                            qsq = sq.tile([P, SB], bf16, name="sq_t")
                            nc.scalar.activation(qsq, tr, AF.Square)
                            qsqs.append(qsq)
                        deferred.append(
                            make_stats(name, sb, trs, qsqs, wst, dst, eps_t, sc))
                        if len(deferred) > 1:
                            deferred.pop(0)()

                # ---- v projection (natural layout, + ones column) ----
                for mc in range(NKV):
                    pv = pj.tile([P, SB], f32, name="pj_t")[:, :DL]
                    for k in range(KC):
                        nc.tensor.matmul(
                            pv,
                            xt[k][:, mc * P:(mc + 1) * P],
                            wv_sb[k],
                            start=(k == 0), stop=(k == KC - 1),
                        )
                    nc.vector.tensor_add(
                        vaug_sb[:, mc, :, 0:HD],
                        pv.rearrange("p (h d) -> p h d", d=HD),
                        vb_bc.rearrange("p (h d) -> p h d", d=HD),
                    )
                    if mc == 0:
                        run_deferred()

            # ================= phase 2: attention + out-projection =========
            # Unit = (qb, h): 16 QK matmuls -> DVE evicts scores to SBUF
            # fp32 -> ONE [128, 4096] exp (ACT, bf16 out) -> 16 AV matmuls.
            # AV for unit U is issued during unit U+1 so the PE FIFO never
            # waits on the exp; the out-projection for q-block qb is issued
            # right after AV(qb, h=3).
            with tc.tile_pool(name="qk", bufs=2, space="PSUM") as qk, \
                 tc.tile_pool(name="av", bufs=2, space="PSUM") as avp, \
                 tc.tile_pool(name="op", bufs=2, space="PSUM") as op, \
                 tc.tile_pool(name="exo", bufs=10) as exo_pool, \
                 tc.tile_pool(name="ev2", bufs=4) as ev2:

                deferred2 = []

                def make_av(qb, h, exos):
                    c, po = h // 2, (h % 2) * HD

                    def emit():
                        av = avp.tile([HD + 1, QB], f32, name="av_t")
                        for j in range(NKV):
                            nc.tensor.matmul(
                                av,
                                vaug_sb[:, j, h, :],
                                exos[j // 4][:, j % 4, :],
                                start=(j == 0), stop=(j == NKV - 1),
                            )
                        srow = ev2.tile([1, QB], f32, name="srow")
                        nc.vector.tensor_copy(srow, av[HD:HD + 1, :])
                        sbc = ev2.tile([HD, QB], f32, name="sbc")
                        nc.gpsimd.dma_start(
                            sbc, srow[0:1, None, :].to_broadcast((1, HD, QB)))
                        rbc = ev2.tile([HD, QB], f32, name="rbc")
                        recip(nc, rbc, sbc)
                        nc.vector.tensor_mul(
                            attT_sb[po:po + HD, c, qb * QB:(qb + 1) * QB],
                            av[0:HD, :], rbc)
                    return emit

                def make_oproj(qb):
                    def emit():
                        for mm in range(QB // P):
                            m = qb * (QB // P) + mm
                            for nb in range(D // SB):
                                pon = op.tile([P, SB], f32, name="op_t")
                                for c in range(CL):
                                    nc.tensor.matmul(
                                        pon,
                                        attT_sb[:, c, m * P:(m + 1) * P],
                                        wo_sb[:, c, nb * SB:(nb + 1) * SB],
                                        start=(c == 0), stop=(c == CL - 1),
                                    )
                                osb = ev2.tile([P, SB], f32, name="osb")
                                nc.vector.tensor_copy(osb, pon)
                                nc.sync.dma_start(
                                    out_d[m, :, nb * SB:(nb + 1) * SB], osb)
                    return emit

                def emit_qk_group(qb, h, jp, exos):
                    c, po = h // 2, (h % 2) * HD
                    sc4 = qk.tile([P, 4, QB], f32, name="qk_t")
                    for jj in range(4):
                        j = jp * 4 + jj
                        nc.tensor.matmul(
                            sc4[:, jj, :],
                            kT_sb[po:po + HD, c, j * P:(j + 1) * P],
                            qTs_sb[po:po + HD, c, qb * QB:(qb + 1) * QB],
                            start=True, stop=True,
                        )
                    exo = exo_pool.tile([P, 4, QB], bf16, name="exo_t")
                    nc.scalar.activation(exo, sc4, AF.Exp)
                    exos.append(exo)

                for qb in range(NQB):
                    for h in range(HL):
                        exos = []
                        for jp in range(2):
                            emit_qk_group(qb, h, jp, exos)
                        # mid-unit: previous unit's AV (+ pending oproj) keeps
                        # the PE fed while this unit's exp drains its PSUM
                        popped_av = False
                        while deferred2 and not popped_av:
                            popped_av = deferred2[0].av
                            deferred2.pop(0).emit()
                        for jp in range(2, 4):
                            emit_qk_group(qb, h, jp, exos)
                        deferred2.append(_Thunk(make_av(qb, h, exos), True))
                        if h == HL - 1:
                            deferred2.append(_Thunk(make_oproj(qb), False))
                while deferred2:
                    deferred2.pop(0).emit()

    nc.compile()
    return nc


def _prepare_core_inputs(inputs):
    """Fold LN centering/gain into weights; shard per core; cast to bf16."""
    import ml_dtypes

    bf = ml_dtypes.bfloat16
    q = np.asarray(inputs["query"], np.float32)
    q_w = np.asarray(inputs["q_w"], np.float64)
    k_w = np.asarray(inputs["k_w"], np.float64)
    v_w = np.asarray(inputs["v_w"], np.float32)
    o_w = np.asarray(inputs["o_w"], np.float32)
    q_b = np.asarray(inputs["q_b"], np.float64)
    k_b = np.asarray(inputs["k_b"], np.float64)
    v_b = np.asarray(inputs["v_b"], np.float32)
    q_g = np.asarray(inputs["q_ln_g"], np.float64)
    k_g = np.asarray(inputs["k_ln_g"], np.float64)

    def fold(w, b, g):
        # per head block (64 out-dims): center across the block, scale by g
        w = w.reshape(H, HD, D)
        w = (w - w.mean(axis=1, keepdims=True)) * g[None, :, None]
        b = b.reshape(H, HD)
        b = (b - b.mean(axis=1, keepdims=True)) * g[None, :]
        return w.reshape(D, D).astype(np.float32), b.reshape(D).astype(np.float32)

    wq_f, qb_f = fold(q_w, q_b, q_g)
    wk_f, kb_f = fold(k_w, k_b, k_g)

    def stat_w(g):
        # w_dd = 1/(64*g_d^2), laid out [CL, P, HL] block-diagonal
        w = np.zeros((DL, HL), np.float64)
        for h in range(HL):
            w[h * HD:(h + 1) * HD, h] = 1.0 / (HD * g[:HD] ** 2)
        return w.reshape(CL, P, HL).astype(bf)

    wsq = stat_w(np.asarray(inputs["q_ln_g"], np.float64))
    wsk = stat_w(np.asarray(inputs["k_ln_g"], np.float64))

    in_maps = []
    for c in range(NCORES):
        b, g = divmod(c, GPC)
        rows = slice(g * DL, (g + 1) * DL)
        in_maps.append({
            "xT": np.ascontiguousarray(q[b].T).reshape(KC, P, S).astype(bf),
            "wqT": np.ascontiguousarray(wq_f[rows].T).reshape(KC, P, DL).astype(bf),
            "wkT": np.ascontiguousarray(wk_f[rows].T).reshape(KC, P, DL).astype(bf),
            "wvT": np.ascontiguousarray(v_w[rows].T).reshape(KC, P, DL).astype(bf),
            "woT": np.ascontiguousarray(o_w[:, rows].T).reshape(CL, P, D).astype(bf),
            "qb": np.ascontiguousarray(qb_f[rows]).reshape(CL, P, 1),
            "kb": np.ascontiguousarray(kb_f[rows]).reshape(CL, P, 1),
            "vb": np.ascontiguousarray(v_b[rows]).reshape(1, DL),
            "wsq": wsq,
            "wsk": wsk,
        })
    return in_maps


def _install_ntff_shim():
    """The agent image's antenv lacks axon_hooks; recreate it so
    run_bass_kernel_spmd(trace=True) can capture NTFF profiles."""
    import types

    try:
        import antenv.axon_hooks  # noqa: F401
        return
    except ImportError:
        pass
    import antenv
    mod = types.ModuleType("antenv.axon_hooks")
    mod._hook = None
    mod.set_axon_ntff_profile_hook = lambda h: setattr(mod, "_hook", h)
    mod.get_axon_ntff_profile_hook = lambda: mod._hook
    sys.modules["antenv.axon_hooks"] = mod
    antenv.axon_hooks = mod
    try:
        from trn_agent_boot.trn_boot import _ntff_profile_via_ctypes
        hook = _ntff_profile_via_ctypes("/opt/axon/libaxon_pjrt.so")
        if hook is not None:
            mod.set_axon_ntff_profile_hook(hook)
    except Exception as e:
        print(f"ntff shim: hook install failed: {e}", file=sys.stderr)


def kernel(**inputs):
    import concourse.bass_utils as bass_utils
    from concourse.bass_utils import run_bass_kernel_spmd

    if "nc" not in _CACHE:
        _CACHE["nc"] = _build_nc()
    nc = _CACHE["nc"]

    in_maps = _prepare_core_inputs(inputs)
    trace = os.environ.get("TRNK_TRACE", "0") == "1"
    if trace:
        _install_ntff_shim()
        # no S3 in this container; keep artifacts local
        bass_utils.upload_artifacts = lambda d: d
    res = run_bass_kernel_spmd(nc, in_maps, core_ids=list(range(NCORES)),
                               trace=trace)
    _CACHE["last_results"] = res

    o_b = np.asarray(inputs["o_b"], np.float32)
    out = np.zeros((B, S, D), np.float32)
    for c in range(NCORES):
        b = c // GPC
        out[b] += res.results[c]["out"].reshape(S, D)
    out += o_b[None, None, :]
    return out


# revision 14
# speedup vs baseline: 1.1973x; 1.1973x over previous
"""Trainium2 Bass kernel: multi-head attention with per-head QK LayerNorm.

Problem shapes: B=2, S=2048, D=1024, H=16 heads, head_dim=64, fp32 I/O.

Sharding (8 cores): core c handles batch b = c//4 and head-group g = c%4
(4 heads = 256 qkv dims). Each core computes its heads' attention and a
partial out-projection; the host sums the 4 partials per batch entry
(tensor-parallel all-reduce done on host at unshard time) and adds o_b.

Algebraic restructurings (all exact modulo fp rounding):
  - LN mean subtraction and gain g are linear => folded into q_w/k_w
    (and biases) on the host.
  - LN variance = sum(w_d * qg_d^2) with w_d = 1/(64*g_d^2): computed on
    device from qg^2 via small matmuls with block-diagonal weights.
  - rstd_q folded into qT columns and tau*rstd_k into kT columns, so
    softmax is a bare exp() of raw scores.  Scores are computed
    TRANSPOSED [kv on partitions, q on free] feeding AV directly.
  - max-subtraction skipped: post-LN rows have norm 8 => |scores| <= 8.
  - sum(exp) over kv falls out of the AV matmul via a ones-column
    appended to V; normalization happens on attT eviction.

Performance notes (from NTFF profiles of earlier versions; this chip
runs a utilization throttler that clamps the PE clock to 1.2 GHz for
half the run under sustained 8-core load, so per-instruction overheads
count double):
  - ALL matmul operands bf16 (fp32 PSUM accumulate).  f32r measured
    2-4x slower on HW (no fast-weight-load, more cold cycles).
  - Phase 1 LN pipeline is BATCHED: projections+evictions+squares+stats
    matmuls stream per block, but sqrt->recip->broadcast->scale-multiply
    run once per q/k tensor at the end (the per-block chain serialized
    the strict-FIFO DVE queue against the DMA queue and stalled the PE
    ~17us per block).
  - Scale-multiplies and their broadcast DMAs overlap the v-projection.
  - exp() reads scores from PSUM in [128, 1024] groups writing bf16
    probs to SBUF (a DVE PSUM->SBUF eviction measured ~1 elem/cycle/lane
    and became the bottleneck when tried).
  - Attention q-block QB=512: halves matmul/LDWEIGHTS/semaphore count
    vs QB=256 at the same cycle count.
  - AV matmuls are software-pipelined one (q-block, head) unit behind
    QK, emitted MID-unit so the strict-FIFO PE queue has work while the
    exp drains this unit's score PSUM banks.
  - Input DMA: biases/stat-weights first (they gate the first eviction
    chain), then x slices alternating between the Sync and Activation
    DGE queues (2 rings in parallel); wv/wo after x (needed later).
  - Partition-broadcast DMAs ride the idle GpSimd DGE queue.
"""

import os
import sys

import numpy as np

for _p in ("/opt/trn_rl_repo",):
    if _p not in sys.path:
        sys.path.append(_p)

# ---- problem constants (hardcoded; kernel.py must be self-contained) ----
B, S, D, H, HD = 2, 2048, 1024, 16, 64
EPS = 1e-5
NCORES = 8
GPC = 4            # cores per batch entry (head-groups)
HL = H // GPC      # 4 local heads
DL = HL * HD       # 256 local qkv dims
P = 128
KC = D // P        # 8 contraction chunks for projections
CL = DL // P       # 2 local-dim partition chunks
SB = 512           # projection free-dim block
NSB = S // SB      # 4 blocks
NKV = S // P       # 16 kv chunks
QB = 512           # attention q-block
NQB = S // QB      # 4 q-blocks

_CACHE = {}


class _Thunk:
    __slots__ = ("emit", "av")

    def __init__(self, emit, av):
        self.emit = emit
        self.av = av


def _build_nc():
    """Build the (single, SPMD-shared) Bass program for one core."""
    import concourse.bass as bass
    import concourse.mybir as mybir
    import concourse.tile as tile
    from concourse import bacc
    from concourse.dve_ops import RECIPROCAL_APPROX_FAST, RECIP_APPROX_FAST_CONSTS

    f32 = mybir.dt.float32
    bf16 = mybir.dt.bfloat16
    AF = mybir.ActivationFunctionType
    rc = RECIP_APPROX_FAST_CONSTS

    def recip(nc, out, in_):
        # ~51-ULP reciprocal in a single DVE pass (vs ~6 cyc/elem exact).
        return nc.vector._custom_dve(
            RECIPROCAL_APPROX_FAST, out=out, in0=in_,
            s0=rc["s0"], s1=rc["s1"], imm2=rc["imm2"],
        )

    nc = bacc.Bacc(trn_type="TRN2")

    xT_d = nc.dram_tensor("xT", [KC, P, S], bf16, kind="ExternalInput")
    wqT_d = nc.dram_tensor("wqT", [KC, P, DL], bf16, kind="ExternalInput")
    wkT_d = nc.dram_tensor("wkT", [KC, P, DL], bf16, kind="ExternalInput")
    wvT_d = nc.dram_tensor("wvT", [KC, P, DL], bf16, kind="ExternalInput")
    woT_d = nc.dram_tensor("woT", [CL, P, D], bf16, kind="ExternalInput")
    qb_d = nc.dram_tensor("qb", [CL, P, 1], f32, kind="ExternalInput")
    kb_d = nc.dram_tensor("kb", [CL, P, 1], f32, kind="ExternalInput")
    vb_d = nc.dram_tensor("vb", [1, DL], f32, kind="ExternalInput")
    wsq_d = nc.dram_tensor("wsq", [CL, P, HL], bf16, kind="ExternalInput")
    wsk_d = nc.dram_tensor("wsk", [CL, P, HL], bf16, kind="ExternalInput")
    out_d = nc.dram_tensor("out", [NKV, P, D], f32, kind="ExternalOutput")

    with tile.TileContext(nc) as tc:
        with tc.tile_pool(name="big", bufs=1) as big:
            # ---- persistent SBUF.  DMA order matters: small gating
            # tensors first, q/k weights next, then x (alternating DGE
            # queues), then wv/wo which are needed only later. ----
            qb_sb = big.tile([P, CL, 1], f32, name="qb_sb")
            kb_sb = big.tile([P, CL, 1], f32, name="kb_sb")
            for c in range(CL):
                nc.sync.dma_start(qb_sb[:, c, :], qb_d[c])
                nc.sync.dma_start(kb_sb[:, c, :], kb_d[c])
            vb_bc = big.tile([P, DL], f32, name="vb_bc")
            nc.sync.dma_start(vb_bc, vb_d[:].to_broadcast((P, DL)))
            wsq_sb = big.tile([P, CL, HL], bf16, name="wsq_sb")
            wsk_sb = big.tile([P, CL, HL], bf16, name="wsk_sb")
            for c in range(CL):
                nc.sync.dma_start(wsq_sb[:, c, :], wsq_d[c])
                nc.sync.dma_start(wsk_sb[:, c, :], wsk_d[c])

            wq_sb, wk_sb, wv_sb = [], [], []
            for wlist, wd, nm, eng in ((wk_sb, wkT_d, "wk", nc.sync),
                                       (wq_sb, wqT_d, "wq", nc.scalar)):
                for k in range(KC):
                    t = big.tile([P, DL], bf16, name=f"{nm}{k}")
                    eng.dma_start(t, wd[k])
                    wlist.append(t)
            xt = [big.tile([P, S], bf16, name=f"xt{k}") for k in range(KC)]
            for sb in range(NSB):
                for k in range(KC):
                    eng = nc.sync if (k % 2 == 0) else nc.scalar
                    eng.dma_start(
                        xt[k][:, sb * SB:(sb + 1) * SB],
                        xT_d[k][:, sb * SB:(sb + 1) * SB])
            for k in range(KC):
                t = big.tile([P, DL], bf16, name=f"wv{k}")
                nc.sync.dma_start(t, wvT_d[k])
                wv_sb.append(t)
            wo_sb = big.tile([P, CL, D], bf16, name="wo_sb")
            for c in range(CL):
                nc.scalar.dma_start(wo_sb[:, c, :], woT_d[c])

            # pre-LN-scale q/k projections (bf16, persistent until the
            # batched scale-multiply at the end of phase 1)
            trk_sb = big.tile([P, CL, S], bf16, name="trk_sb")
            trq_sb = big.tile([P, CL, S], bf16, name="trq_sb")
            # per-(head, s) reciprocal std, collected for the whole tensor
            rsk_sb = big.tile([HL, S], f32, name="rsk_sb")
            rsq_sb = big.tile([HL, S], f32, name="rsq_sb")
            rrk_sb = big.tile([HL, S], bf16, name="rrk_sb")
            rrq_sb = big.tile([HL, S], bf16, name="rrq_sb")
            qsck_sb = big.tile([P, CL, S], bf16, name="qsck_sb")
            qscq_sb = big.tile([P, CL, S], bf16, name="qscq_sb")

            kT_sb = big.tile([P, CL, S], bf16, name="kT_sb")
            qTs_sb = big.tile([P, CL, S], bf16, name="qTs_sb")
            vaug_sb = big.tile([P, NKV, HL, HD + 1], bf16, name="vaug_sb")
            attT_sb = big.tile([P, CL, S], bf16, name="attT_sb")
            nc.vector.memset(vaug_sb[:, :, :, HD:HD + 1], 1.0)
            eps_q = big.tile([P, 1], f32, name="eps_q")
            nc.vector.memset(eps_q, EPS)
            eps_k = big.tile([P, 1], f32, name="eps_k")
            nc.vector.memset(eps_k, 64.0 * EPS)

            # ============ phase 1: projections + LN stat folding ===========
            with tc.tile_pool(name="pj", bufs=3, space="PSUM") as pj, \
                 tc.tile_pool(name="st", bufs=2, space="PSUM") as st, \
                 tc.tile_pool(name="sq", bufs=4) as sq:

                deferred = []

                def make_stats(sb, qsqs, wst, rs_dst, eps_t, sc):
                    def emit():
                        stp = st.tile([HL, SB], f32, name="st_t")
                        for c in range(CL):
                            nc.tensor.matmul(
                                stp, wst[:, c, :], qsqs[c],
                                start=(c == 0), stop=(c == CL - 1),
                            )
                        # per-block sqrt straight into the per-tensor
                        # std collection tile (ACT; the recip +
                        # broadcast + multiply are batched later)
                        nc.scalar.activation(
                            rs_dst[:, sb * SB:(sb + 1) * SB], stp, AF.Sqrt,
                            bias=eps_t[:HL], scale=sc)
                    return emit

                for name, wlist, bcol, wst, tr_dst, rs_dst, eps_t, sc in (
                        ("k", wk_sb, kb_sb, wsk_sb, trk_sb, rsk_sb, eps_k, 64.0),
                        ("q", wq_sb, qb_sb, wsq_sb, trq_sb, rsq_sb, eps_q, 1.0)):
                    for sb in range(NSB):
                        qsqs = []
                        for c in range(CL):
                            ph = pj.tile([P, SB], f32, name="pj_t")
                            for k in range(KC):
                                nc.tensor.matmul(
                                    ph, wlist[k][:, c * P:(c + 1) * P],
                                    xt[k][:, sb * SB:(sb + 1) * SB],
                                    start=(k == 0), stop=(k == KC - 1),
                                )
                            tr = tr_dst[:, c, sb * SB:(sb + 1) * SB]
                            nc.vector.tensor_scalar_add(tr, ph, bcol[:, c, :])
                            qsq = sq.tile([P, SB], bf16, name="sq_t")
                            nc.scalar.activation(qsq, tr, AF.Square)
                            qsqs.append(qsq)
                        deferred.append(
                            make_stats(sb, qsqs, wst, rs_dst, eps_t, sc))
                        if len(deferred) > 1:
                            deferred.pop(0)()
                    if name == "k":
                        # k stats tail: emitted before the q projections so
                        # kT is ready well before the first QK matmul.
                        deferred.pop(0)()

                        def scale_tail(rs, rr, qsc, tr_t, dst):
                            recip(nc, rr, rs)
                            for c in range(CL):
                                nc.gpsimd.dma_start(
                                    qsc[:, c, :],
                                    rr[c * 2:(c + 1) * 2, None, :]
                                    .to_broadcast((2, HD, S)),
                                )
                                nc.vector.tensor_mul(
                                    dst[:, c, :], tr_t[:, c, :], qsc[:, c, :])

                        scale_tail(rsk_sb, rrk_sb, qsck_sb, trk_sb, kT_sb)

                # flush q's last stats, then its scale tail
                deferred.pop(0)()
                scale_tail(rsq_sb, rrq_sb, qscq_sb, trq_sb, qTs_sb)

                # ---- v projection (natural layout, + ones column) ----
                # runs on the PE while the q/k scale tails execute on
                # DVE/GpSimd-DMA.
                for mc in range(NKV):
                    pv = pj.tile([P, SB], f32, name="pj_t")[:, :DL]
                    for k in range(KC):
                        nc.tensor.matmul(
                            pv,
                            xt[k][:, mc * P:(mc + 1) * P],
                            wv_sb[k],
                            start=(k == 0), stop=(k == KC - 1),
                        )
                    nc.vector.tensor_add(
                        vaug_sb[:, mc, :, 0:HD],
                        pv.rearrange("p (h d) -> p h d", d=HD),
                        vb_bc.rearrange("p (h d) -> p h d", d=HD),
                    )

            # ================= phase 2: attention + out-projection =========
            # Unit = (qb, h) with QB=512: 16 QK matmuls (N=512) in pairs,
            # exp per [128, 1024] PSUM group -> bf16 SBUF, 16 AV matmuls.
            # AV for unit U is issued MID-unit U+1 so the PE FIFO never
            # waits on the exp; out-projection per q-block follows its
            # last head's AV.
            with tc.tile_pool(name="qk", bufs=2, space="PSUM") as qk, \
                 tc.tile_pool(name="av", bufs=2, space="PSUM") as avp, \
                 tc.tile_pool(name="op", bufs=2, space="PSUM") as op, \
                 tc.tile_pool(name="exo", bufs=12) as exo_pool, \
                 tc.tile_pool(name="ev2", bufs=4) as ev2:

                deferred2 = []

                def make_av(qb, h, exos):
                    c, po = h // 2, (h % 2) * HD

                    def emit():
                        av = avp.tile([HD + 1, QB], f32, name="av_t")
                        for j in range(NKV):
                            nc.tensor.matmul(
                                av,
                                vaug_sb[:, j, h, :],
                                exos[j // 2][:, j % 2, :],
                                start=(j == 0), stop=(j == NKV - 1),
                            )
                        srow = ev2.tile([1, QB], f32, name="srow")
                        nc.vector.tensor_copy(srow, av[HD:HD + 1, :])
                        sbc = ev2.tile([HD, QB], f32, name="sbc")
                        nc.gpsimd.dma_start(
                            sbc, srow[0:1, None, :].to_broadcast((1, HD, QB)))
                        rbc = ev2.tile([HD, QB], f32, name="rbc")
                        recip(nc, rbc, sbc)
                        nc.vector.tensor_mul(
                            attT_sb[po:po + HD, c, qb * QB:(qb + 1) * QB],
                            av[0:HD, :], rbc)
                    return emit

                def make_oproj(qb):
                    def emit():
                        for mm in range(QB // P):
                            m = qb * (QB // P) + mm
                            for nb in range(D // SB):
                                pon = op.tile([P, SB], f32, name="op_t")
                                for c in range(CL):
                                    nc.tensor.matmul(
                                        pon,
                                        attT_sb[:, c, m * P:(m + 1) * P],
                                        wo_sb[:, c, nb * SB:(nb + 1) * SB],
                                        start=(c == 0), stop=(c == CL - 1),
                                    )
                                osb = ev2.tile([P, SB], f32, name="osb")
                                nc.vector.tensor_copy(osb, pon)
                                nc.sync.dma_start(
                                    out_d[m, :, nb * SB:(nb + 1) * SB], osb)
                    return emit

                def emit_qk_group(qb, h, jp, exos):
                    c, po = h // 2, (h % 2) * HD
                    sc2 = qk.tile([P, 2, QB], f32, name="qk_t")
                    for jj in range(2):
                        j = jp * 2 + jj
                        nc.tensor.matmul(
                            sc2[:, jj, :],
                            kT_sb[po:po + HD, c, j * P:(j + 1) * P],
                            qTs_sb[po:po + HD, c, qb * QB:(qb + 1) * QB],
                            start=True, stop=True,
                        )
                    exo = exo_pool.tile([P, 2, QB], bf16, name="exo_t")
                    nc.scalar.activation(exo, sc2, AF.Exp)
                    exos.append(exo)

                for qb in range(NQB):
                    for h in range(HL):
                        exos = []
                        for jp in range(4):
                            emit_qk_group(qb, h, jp, exos)
                        # mid-unit: previous unit's AV (+ pending oproj)
                        # keeps the PE fed while this unit's exp drains
                        # its PSUM banks
                        popped_av = False
                        while deferred2 and not popped_av:
                            popped_av = deferred2[0].av
                            deferred2.pop(0).emit()
                        for jp in range(4, 8):
                            emit_qk_group(qb, h, jp, exos)
                        deferred2.append(_Thunk(make_av(qb, h, exos), True))
                        if h == HL - 1:
                            deferred2.append(_Thunk(make_oproj(qb), False))
                while deferred2:
                    deferred2.pop(0).emit()

    nc.compile()
    return nc


def _prepare_core_inputs(inputs):
    """Fold LN centering/gain into weights; shard per core; cast to bf16."""
    import ml_dtypes

    bf = ml_dtypes.bfloat16
    q = np.asarray(inputs["query"], np.float32)
    q_w = np.asarray(inputs["q_w"], np.float64)
    k_w = np.asarray(inputs["k_w"], np.float64)
    v_w = np.asarray(inputs["v_w"], np.float32)
    o_w = np.asarray(inputs["o_w"], np.float32)
    q_b = np.asarray(inputs["q_b"], np.float64)
    k_b = np.asarray(inputs["k_b"], np.float64)
    v_b = np.asarray(inputs["v_b"], np.float32)
    q_g = np.asarray(inputs["q_ln_g"], np.float64)
    k_g = np.asarray(inputs["k_ln_g"], np.float64)

    def fold(w, b, g):
        # per head block (64 out-dims): center across the block, scale by g
        w = w.reshape(H, HD, D)
        w = (w - w.mean(axis=1, keepdims=True)) * g[None, :, None]
        b = b.reshape(H, HD)
        b = (b - b.mean(axis=1, keepdims=True)) * g[None, :]
        return w.reshape(D, D).astype(np.float32), b.reshape(D).astype(np.float32)

    wq_f, qb_f = fold(q_w, q_b, q_g)
    wk_f, kb_f = fold(k_w, k_b, k_g)

    def stat_w(g):
        # w_dd = 1/(64*g_d^2), laid out [CL, P, HL] block-diagonal
        w = np.zeros((DL, HL), np.float64)
        for h in range(HL):
            w[h * HD:(h + 1) * HD, h] = 1.0 / (HD * g[:HD] ** 2)
        return w.reshape(CL, P, HL).astype(bf)

    wsq = stat_w(np.asarray(inputs["q_ln_g"], np.float64))
    wsk = stat_w(np.asarray(inputs["k_ln_g"], np.float64))

    in_maps = []
    for c in range(NCORES):
        b, g = divmod(c, GPC)
        rows = slice(g * DL, (g + 1) * DL)
        in_maps.append({
            "xT": np.ascontiguousarray(q[b].T).reshape(KC, P, S).astype(bf),
            "wqT": np.ascontiguousarray(wq_f[rows].T).reshape(KC, P, DL).astype(bf),
            "wkT": np.ascontiguousarray(wk_f[rows].T).reshape(KC, P, DL).astype(bf),
            "wvT": np.ascontiguousarray(v_w[rows].T).reshape(KC, P, DL).astype(bf),
            "woT": np.ascontiguousarray(o_w[:, rows].T).reshape(CL, P, D).astype(bf),
            "qb": np.ascontiguousarray(qb_f[rows]).reshape(CL, P, 1),
            "kb": np.ascontiguousarray(kb_f[rows]).reshape(CL, P, 1),
            "vb": np.ascontiguousarray(v_b[rows]).reshape(1, DL),
            "wsq": wsq,
            "wsk": wsk,
        })
    return in_maps


def _install_ntff_shim():
    """The agent image's antenv lacks axon_hooks; recreate it so
    run_bass_kernel_spmd(trace=True) can capture NTFF profiles."""
    import types

    try:
        import antenv.axon_hooks  # noqa: F401
        return
    except ImportError:
        pass
    import antenv
    mod = types.ModuleType("antenv.axon_hooks")
    mod._hook = None
    mod.set_axon_ntff_profile_hook = lambda h: setattr(mod, "_hook", h)
    mod.get_axon_ntff_profile_hook = lambda: mod._hook
    sys.modules["antenv.axon_hooks"] = mod
    antenv.axon_hooks = mod
    try:
        from trn_agent_boot.trn_boot import _ntff_profile_via_ctypes
        hook = _ntff_profile_via_ctypes("/opt/axon/libaxon_pjrt.so")
        if hook is not None:
            mod.set_axon_ntff_profile_hook(hook)
    except Exception as e:
        print(f"ntff shim: hook install failed: {e}", file=sys.stderr)


def kernel(**inputs):
    import concourse.bass_utils as bass_utils
    from concourse.bass_utils import run_bass_kernel_spmd

    if "nc" not in _CACHE:
        _CACHE["nc"] = _build_nc()
    nc = _CACHE["nc"]

    in_maps = _prepare_core_inputs(inputs)
    trace = os.environ.get("TRNK_TRACE", "0") == "1"
    if trace:
        _install_ntff_shim()
        # no S3 in this container; keep artifacts local
        bass_utils.upload_artifacts = lambda d: d
    res = run_bass_kernel_spmd(nc, in_maps, core_ids=list(range(NCORES)),
                               trace=trace)
    _CACHE["last_results"] = res

    o_b = np.asarray(inputs["o_b"], np.float32)
    out = np.zeros((B, S, D), np.float32)
    for c in range(NCORES):
        b = c // GPC
        out[b] += res.results[c]["out"].reshape(S, D)
    out += o_b[None, None, :]
    return out


# revision 32
# speedup vs baseline: 1.6367x; 1.3671x over previous
"""Trainium2 Bass kernel: multi-head attention with per-head QK LayerNorm.

Problem shapes: B=2, S=2048, D=1024, H=16 heads, head_dim=64, fp32 I/O.

Sharding (8 cores): core c handles batch b = c//4 and head-group g = c%4
(4 heads = 256 qkv dims). Each core computes its heads' attention and a
partial out-projection; the host sums the 4 partials per batch entry
(tensor-parallel all-reduce done on host at unshard time) and adds o_b.

Algebraic restructurings (all exact modulo fp rounding):
  - LN mean subtraction and gain g are linear => folded into q_w/k_w
    (and biases) on the host.
  - LN variance = sum(w_d * qg_d^2) with w_d = 1/(64*g_d^2): computed on
    device from qg^2 via small matmuls with block-diagonal weights.
  - rstd_q folded into qT columns and tau*rstd_k into kT columns, so
    softmax is a bare exp() of raw scores.  Scores are computed
    TRANSPOSED [kv on partitions, q on free] feeding AV directly.
  - max-subtraction skipped: post-LN rows have norm 8 => |scores| <= 8.
  - sum(exp) over kv falls out of the AV matmul via a ones-column
    appended to V; normalization happens on attT eviction.

Performance notes (from NTFF profiles of earlier versions; this chip
runs a utilization throttler that clamps the PE clock to 1.2 GHz for
half the run under sustained 8-core load, so per-instruction overheads
count double):
  - ALL matmul operands bf16 (fp32 PSUM accumulate).  f32r measured
    2-4x slower on HW (no fast-weight-load, more cold cycles).
  - Phase 1 LN pipeline is BATCHED: projections+evictions+squares+stats
    matmuls stream per block, but sqrt->recip->broadcast->scale-multiply
    run once per q/k tensor at the end (the per-block chain serialized
    the strict-FIFO DVE queue against the DMA queue and stalled the PE
    ~17us per block).
  - Scale-multiplies and their broadcast DMAs overlap the v-projection.
  - exp() reads scores from PSUM in [128, 1024] groups writing bf16
    probs to SBUF (a DVE PSUM->SBUF eviction measured ~1 elem/cycle/lane
    and became the bottleneck when tried).
  - Attention q-block QB=512: halves matmul/LDWEIGHTS/semaphore count
    vs QB=256 at the same cycle count.
  - AV matmuls are software-pipelined one (q-block, head) unit behind
    QK, emitted MID-unit so the strict-FIFO PE queue has work while the
    exp drains this unit's score PSUM banks.
  - Input DMA: biases/stat-weights first (they gate the first eviction
    chain), then x slices alternating between the Sync and Activation
    DGE queues (2 rings in parallel); wv/wo after x (needed later).
  - Partition-broadcast DMAs ride the idle GpSimd DGE queue.
"""

import os
import sys

import numpy as np

for _p in ("/opt/trn_rl_repo",):
    if _p not in sys.path:
        sys.path.append(_p)

# ---- problem constants (hardcoded; kernel.py must be self-contained) ----
B, S, D, H, HD = 2, 2048, 1024, 16, 64
EPS = 1e-5
NCORES = 8
GPC = 4            # cores per batch entry (head-groups)
HL = H // GPC      # 4 local heads
DL = HL * HD       # 256 local qkv dims
P = 128
KC = D // P        # 8 contraction chunks for projections
CL = DL // P       # 2 local-dim partition chunks
SB = 512           # projection free-dim block
NSB = S // SB      # 4 blocks
NKV = S // P       # 16 kv chunks
QB = 512           # attention q-block
NQB = S // QB      # 4 q-blocks

_CACHE = {}


class _Thunk:
    __slots__ = ("emit", "av")

    def __init__(self, emit, av):
        self.emit = emit
        self.av = av


def _build_nc():
    """Build the (single, SPMD-shared) Bass program for one core."""
    import concourse.bass as bass
    import concourse.mybir as mybir
    import concourse.tile as tile
    from concourse import bacc
    from concourse.dve_ops import RECIPROCAL_APPROX_FAST, RECIP_APPROX_FAST_CONSTS

    f32 = mybir.dt.float32
    bf16 = mybir.dt.bfloat16
    AF = mybir.ActivationFunctionType
    rc = RECIP_APPROX_FAST_CONSTS

    def recip(nc, out, in_):
        # ~51-ULP reciprocal in a single DVE pass (vs ~6 cyc/elem exact).
        return nc.vector._custom_dve(
            RECIPROCAL_APPROX_FAST, out=out, in0=in_,
            s0=rc["s0"], s1=rc["s1"], imm2=rc["imm2"],
        )

    nc = bacc.Bacc(trn_type="TRN2")

    xT_d = nc.dram_tensor("xT", [KC, P, S], bf16, kind="ExternalInput")
    wqT_d = nc.dram_tensor("wqT", [KC, P, DL], bf16, kind="ExternalInput")
    wkT_d = nc.dram_tensor("wkT", [KC, P, DL], bf16, kind="ExternalInput")
    wvT_d = nc.dram_tensor("wvT", [KC, P, DL], bf16, kind="ExternalInput")
    woT_d = nc.dram_tensor("woT", [CL, P, D], bf16, kind="ExternalInput")
    qb_d = nc.dram_tensor("qb", [CL, P, 1], f32, kind="ExternalInput")
    kb_d = nc.dram_tensor("kb", [CL, P, 1], f32, kind="ExternalInput")
    vb_d = nc.dram_tensor("vb", [1, DL], f32, kind="ExternalInput")
    wsq_d = nc.dram_tensor("wsq", [CL, P, HL], bf16, kind="ExternalInput")
    wsk_d = nc.dram_tensor("wsk", [CL, P, HL], bf16, kind="ExternalInput")
    ones2_d = nc.dram_tensor("ones2", [2, P], bf16, kind="ExternalInput")
    out_d = nc.dram_tensor("out", [NKV, P, D], f32, kind="ExternalOutput")

    with tile.TileContext(nc) as tc:
        with tc.tile_pool(name="big", bufs=1) as big:
            # ---- persistent SBUF.  DMA order matters: small gating
            # tensors first, q/k weights next, then x (alternating DGE
            # queues), then wv/wo which are needed only later. ----
            qb_sb = big.tile([P, CL, 1], f32, name="qb_sb")
            kb_sb = big.tile([P, CL, 1], f32, name="kb_sb")
            for c in range(CL):
                nc.sync.dma_start(qb_sb[:, c, :], qb_d[c])
                nc.sync.dma_start(kb_sb[:, c, :], kb_d[c])
            vb_bc = big.tile([P, DL], f32, name="vb_bc")
            nc.sync.dma_start(vb_bc, vb_d[:].to_broadcast((P, DL)))
            wsq_sb = big.tile([P, CL, HL], bf16, name="wsq_sb")
            wsk_sb = big.tile([P, CL, HL], bf16, name="wsk_sb")
            for c in range(CL):
                nc.sync.dma_start(wsq_sb[:, c, :], wsq_d[c])
                nc.sync.dma_start(wsk_sb[:, c, :], wsk_d[c])

            wq_sb, wk_sb, wv_sb = [], [], []
            for wlist, wd, nm, eng in ((wk_sb, wkT_d, "wk", nc.sync),
                                       (wq_sb, wqT_d, "wq", nc.scalar)):
                for k in range(KC):
                    t = big.tile([P, DL], bf16, name=f"{nm}{k}")
                    eng.dma_start(t, wd[k])
                    wlist.append(t)
            xt = [big.tile([P, S], bf16, name=f"xt{k}") for k in range(KC)]
            for sb in range(NSB):
                for k in range(KC):
                    eng = nc.sync if (k % 2 == 0) else nc.scalar
                    eng.dma_start(
                        xt[k][:, sb * SB:(sb + 1) * SB],
                        xT_d[k][:, sb * SB:(sb + 1) * SB])
            for k in range(KC):
                t = big.tile([P, DL], bf16, name=f"wv{k}")
                nc.sync.dma_start(t, wvT_d[k])
                wv_sb.append(t)
            wo_sb = big.tile([P, CL, D], bf16, name="wo_sb")
            for c in range(CL):
                nc.scalar.dma_start(wo_sb[:, c, :], woT_d[c])

            # pre-LN-scale q/k projections (bf16, persistent until the
            # batched scale-multiply at the end of phase 1)
            trk_sb = big.tile([P, CL, S], bf16, name="trk_sb")
            trq_sb = big.tile([P, CL, S], bf16, name="trq_sb")
            # per-(head, s) reciprocal std, collected for the whole tensor
            # per-c std / reciprocal-std rows (partition base 0: the DVE /
            # matmul ISA rejects partition offsets outside {0,32,64,96})
            rs_t = {(n, c): big.tile([2, S], f32, name=f"rs_{n}{c}")
                    for n in ("k", "q") for c in range(CL)}
            rr_t = {(n, c): big.tile([2, S], bf16, name=f"rr_{n}{c}")
                    for n in ("k", "q") for c in range(CL)}
            # ones patterns for PE partition-broadcast matmuls:
            # ones2[i, p] = 1 iff p//64 == i  (row i -> partitions 64i..64i+63)
            ones2 = big.tile([2, P], bf16, name="ones2")
            nc.sync.dma_start(ones2, ones2_d[:])
            ones1 = big.tile([1, HD], bf16, name="ones1")
            nc.vector.memset(ones1, 1.0)

            kT_sb = big.tile([P, CL, S], bf16, name="kT_sb")
            qTs_sb = big.tile([P, CL, S], bf16, name="qTs_sb")
            vaug_sb = big.tile([P, NKV, HL, HD + 1], bf16, name="vaug_sb")
            attT_sb = big.tile([P, CL, S], bf16, name="attT_sb")
            nc.vector.memset(vaug_sb[:, :, :, HD:HD + 1], 1.0)
            eps_q = big.tile([P, 1], f32, name="eps_q")
            nc.vector.memset(eps_q, EPS)
            eps_k = big.tile([P, 1], f32, name="eps_k")
            nc.vector.memset(eps_k, 64.0 * EPS)

            # ============ phase 1: projections + LN stat folding ===========
            with tc.tile_pool(name="pj", bufs=3, space="PSUM") as pj, \
                 tc.tile_pool(name="st", bufs=4, space="PSUM") as st, \
                 tc.tile_pool(name="sq", bufs=5) as sq:

                deferred = []

                def make_stats(name, sb, qsqs, wst, eps_t, sc):
                    def emit():
                        for c in range(CL):
                            # block-diagonal stats: chunk c only holds
                            # heads 2c, 2c+1
                            stp = st.tile([2, SB], f32, name="st_t")
                            nc.tensor.matmul(
                                stp, wst[:, c, 2 * c:2 * c + 2], qsqs[c],
                                start=True, stop=True,
                            )
                            # per-block sqrt straight into the per-tensor
                            # std collection tile (ACT; the recip +
                            # broadcast + multiply are batched later)
                            nc.scalar.activation(
                                rs_t[(name, c)][:, sb * SB:(sb + 1) * SB],
                                stp, AF.Sqrt, bias=eps_t[:2], scale=sc)
                    return emit

                P1 = {
                    "k": (wk_sb, kb_sb, wsk_sb, trk_sb, eps_k, 64.0),
                    "q": (wq_sb, qb_sb, wsq_sb, trq_sb, eps_q, 1.0),
                }

                def emit_proj(name, sb):
                    wlist, bcol, wst, tr_dst, eps_t, sc = P1[name]
                    qsqs = []
                    for c in range(CL):
                        ph = pj.tile([P, SB], f32, name="pj_t")
                        for k in range(KC):
                            nc.tensor.matmul(
                                ph, wlist[k][:, c * P:(c + 1) * P],
                                xt[k][:, sb * SB:(sb + 1) * SB],
                                start=(k == 0), stop=(k == KC - 1),
                            )
                        tr = tr_dst[:, c, sb * SB:(sb + 1) * SB]
                        nc.vector.tensor_scalar_add(tr, ph, bcol[:, c, :])
                        qsq = sq.tile([P, SB], bf16, name="sq_t")
                        nc.scalar.activation(qsq, tr, AF.Square)
                        qsqs.append(qsq)
                    deferred.append(
                        make_stats(name, sb, qsqs, wst, eps_t, sc))
                    if len(deferred) > 1:
                        deferred.pop(0)()

                def emit_vproj(mc):
                    pv = pj.tile([P, SB], f32, name="pj_t")[:, :DL]
                    for k in range(KC):
                        nc.tensor.matmul(
                            pv,
                            xt[k][:, mc * P:(mc + 1) * P],
                            wv_sb[k],
                            start=(k == 0), stop=(k == KC - 1),
                        )
                    nc.vector.tensor_add(
                        vaug_sb[:, mc, :, 0:HD],
                        pv.rearrange("p (h d) -> p h d", d=HD),
                        vb_bc.rearrange("p (h d) -> p h d", d=HD),
                    )

                def scale_tail(name, dst):
                    # rstd broadcast via PE ones-matmul (a GpSimd broadcast
                    # DMA here measured >20us and stalled the DVE FIFO):
                    # out_psum[p, s] = rr[c][p // 64, s], then one DVE
                    # multiply folds it into kT/qTs.
                    tr_dst = P1[name][3]
                    for c in range(CL):
                        recip(nc, rr_t[(name, c)], rs_t[(name, c)])
                    for c in range(CL):
                        for sb in range(NSB):
                            bp = pj.tile([P, SB], f32, name="pj_t")
                            nc.tensor.matmul(
                                bp, ones2,
                                rr_t[(name, c)][:, sb * SB:(sb + 1) * SB],
                                start=True, stop=True,
                            )
                            nc.vector.tensor_mul(
                                dst[:, c, sb * SB:(sb + 1) * SB],
                                tr_dst[:, c, sb * SB:(sb + 1) * SB], bp)

                for sb in range(NSB):
                    emit_proj("k", sb)
                deferred.pop(0)()          # k sb3 stats
                emit_proj("q", 0)          # PE filler while k recip runs
                scale_tail("k", kT_sb)
                for sb in range(1, NSB):
                    emit_proj("q", sb)
                deferred.pop(0)()          # q sb3 stats
                emit_vproj(0)              # PE filler while q recip runs
                emit_vproj(1)
                scale_tail("q", qTs_sb)
                for mc in range(2, NKV):
                    emit_vproj(mc)

            # ================= phase 2: attention + out-projection =========
            # Unit = (qb, h) with QB=512: 16 QK matmuls (N=512) in pairs,
            # exp per [128, 1024] PSUM group -> bf16 SBUF, 16 AV matmuls.
            # AV for unit U is issued MID-unit U+1 so the PE FIFO never
            # waits on the exp; out-projection per q-block follows its
            # last head's AV.
            with tc.tile_pool(name="qk", bufs=2, space="PSUM") as qk, \
                 tc.tile_pool(name="av", bufs=2, space="PSUM") as avp, \
                 tc.tile_pool(name="op", bufs=1, space="PSUM") as op, \
                 tc.tile_pool(name="exo", bufs=12) as exo_pool, \
                 tc.tile_pool(name="ev2", bufs=3) as ev2:

                deferred2 = []

                def make_av(qb, h, exos):
                    c, po = h // 2, (h % 2) * HD

                    def emit():
                        av = avp.tile([HD + 1, QB], f32, name="av_t")
                        for j in range(NKV):
                            nc.tensor.matmul(
                                av,
                                vaug_sb[:, j, h, :],
                                exos[j // 2][:, j % 2, :],
                                start=(j == 0), stop=(j == NKV - 1),
                            )
                        # softmax denominators: copy the sums row, take the
                        # reciprocal (bf16), broadcast across 64 partitions
                        # with a ones-matmul, evict, multiply.
                        srow = ev2.tile([1, QB], f32, name="srow")
                        nc.vector.tensor_copy(srow, av[HD:HD + 1, :])
                        rb1 = ev2.tile([1, QB], bf16, name="rb1")
                        recip(nc, rb1, srow)
                        rbp = avp.tile([HD, QB], f32, name="rbp", bufs=1)
                        nc.tensor.matmul(rbp, ones1, rb1,
                                         start=True, stop=True)
                        rbc = ev2.tile([HD, QB], f32, name="rbc")
                        nc.vector.tensor_copy(rbc, rbp)
                        nc.vector.tensor_mul(
                            attT_sb[po:po + HD, c, qb * QB:(qb + 1) * QB],
                            av[0:HD, :], rbc)
                    return emit

                def make_oproj(qb):
                    def emit():
                        for mm in range(QB // P):
                            m = qb * (QB // P) + mm
                            for nb in range(D // SB):
                                pon = op.tile([P, SB], f32, name="op_t")
                                for c in range(CL):
                                    nc.tensor.matmul(
                                        pon,
                                        attT_sb[:, c, m * P:(m + 1) * P],
                                        wo_sb[:, c, nb * SB:(nb + 1) * SB],
                                        start=(c == 0), stop=(c == CL - 1),
                                    )
                                osb = ev2.tile([P, SB], f32, name="osb")
                                nc.vector.tensor_copy(osb, pon)
                                nc.sync.dma_start(
                                    out_d[m, :, nb * SB:(nb + 1) * SB], osb)
                    return emit

                def emit_qk_group(qb, h, jp, exos):
                    c, po = h // 2, (h % 2) * HD
                    sc2 = qk.tile([P, 2, QB], f32, name="qk_t")
                    for jj in range(2):
                        j = jp * 2 + jj
                        nc.tensor.matmul(
                            sc2[:, jj, :],
                            kT_sb[po:po + HD, c, j * P:(j + 1) * P],
                            qTs_sb[po:po + HD, c, qb * QB:(qb + 1) * QB],
                            start=True, stop=True,
                        )
                    exo = exo_pool.tile([P, 2, QB], bf16, name="exo_t")
                    nc.scalar.activation(exo, sc2, AF.Exp)
                    exos.append(exo)

                for qb in range(NQB):
                    for h in range(HL):
                        exos = []
                        for jp in range(4):
                            emit_qk_group(qb, h, jp, exos)
                        # mid-unit: previous unit's AV (+ pending oproj)
                        # keeps the PE fed while this unit's exp drains
                        # its PSUM banks
                        popped_av = False
                        while deferred2 and not popped_av:
                            popped_av = deferred2[0].av
                            deferred2.pop(0).emit()
                        for jp in range(4, 8):
                            emit_qk_group(qb, h, jp, exos)
                        deferred2.append(_Thunk(make_av(qb, h, exos), True))
                        if h == HL - 1:
                            deferred2.append(_Thunk(make_oproj(qb), False))
                while deferred2:
                    deferred2.pop(0).emit()

    nc.compile()
    return nc


def _prepare_core_inputs(inputs):
    """Fold LN centering/gain into weights; shard per core; cast to bf16."""
    import ml_dtypes

    bf = ml_dtypes.bfloat16
    q = np.asarray(inputs["query"], np.float32)
    q_w = np.asarray(inputs["q_w"], np.float64)
    k_w = np.asarray(inputs["k_w"], np.float64)
    v_w = np.asarray(inputs["v_w"], np.float32)
    o_w = np.asarray(inputs["o_w"], np.float32)
    q_b = np.asarray(inputs["q_b"], np.float64)
    k_b = np.asarray(inputs["k_b"], np.float64)
    v_b = np.asarray(inputs["v_b"], np.float32)
    q_g = np.asarray(inputs["q_ln_g"], np.float64)
    k_g = np.asarray(inputs["k_ln_g"], np.float64)

    def fold(w, b, g):
        # per head block (64 out-dims): center across the block, scale by g
        w = w.reshape(H, HD, D)
        w = (w - w.mean(axis=1, keepdims=True)) * g[None, :, None]
        b = b.reshape(H, HD)
        b = (b - b.mean(axis=1, keepdims=True)) * g[None, :]
        return w.reshape(D, D).astype(np.float32), b.reshape(D).astype(np.float32)

    wq_f, qb_f = fold(q_w, q_b, q_g)
    wk_f, kb_f = fold(k_w, k_b, k_g)

    def stat_w(g):
        # w_dd = 1/(64*g_d^2), laid out [CL, P, HL] block-diagonal
        w = np.zeros((DL, HL), np.float64)
        for h in range(HL):
            w[h * HD:(h + 1) * HD, h] = 1.0 / (HD * g[:HD] ** 2)
        return w.reshape(CL, P, HL).astype(bf)

    wsq = stat_w(np.asarray(inputs["q_ln_g"], np.float64))
    wsk = stat_w(np.asarray(inputs["k_ln_g"], np.float64))
    # ones2[i, p] = 1 iff p//64 == i: partition-broadcast matmul pattern
    ones2_np = np.zeros((2, P), bf)
    ones2_np[0, :HD] = 1
    ones2_np[1, HD:] = 1

    in_maps = []
    for c in range(NCORES):
        b, g = divmod(c, GPC)
        rows = slice(g * DL, (g + 1) * DL)
        in_maps.append({
            "xT": np.ascontiguousarray(q[b].T).reshape(KC, P, S).astype(bf),
            "wqT": np.ascontiguousarray(wq_f[rows].T).reshape(KC, P, DL).astype(bf),
            "wkT": np.ascontiguousarray(wk_f[rows].T).reshape(KC, P, DL).astype(bf),
            "wvT": np.ascontiguousarray(v_w[rows].T).reshape(KC, P, DL).astype(bf),
            "woT": np.ascontiguousarray(o_w[:, rows].T).reshape(CL, P, D).astype(bf),
            "qb": np.ascontiguousarray(qb_f[rows]).reshape(CL, P, 1),
            "kb": np.ascontiguousarray(kb_f[rows]).reshape(CL, P, 1),
            "vb": np.ascontiguousarray(v_b[rows]).reshape(1, DL),
            "wsq": wsq,
            "wsk": wsk,
            "ones2": ones2_np,
        })
    return in_maps


def _install_ntff_shim():
    """The agent image's antenv lacks axon_hooks; recreate it so
    run_bass_kernel_spmd(trace=True) can capture NTFF profiles."""
    import types

    try:
        import antenv.axon_hooks  # noqa: F401
        return
    except ImportError:
        pass
    import antenv
    mod = types.ModuleType("antenv.axon_hooks")
    mod._hook = None
    mod.set_axon_ntff_profile_hook = lambda h: setattr(mod, "_hook", h)
    mod.get_axon_ntff_profile_hook = lambda: mod._hook
    sys.modules["antenv.axon_hooks"] = mod
    antenv.axon_hooks = mod
    try:
        from trn_agent_boot.trn_boot import _ntff_profile_via_ctypes
        hook = _ntff_profile_via_ctypes("/opt/axon/libaxon_pjrt.so")
        if hook is not None:
            mod.set_axon_ntff_profile_hook(hook)
    except Exception as e:
        print(f"ntff shim: hook install failed: {e}", file=sys.stderr)


def kernel(**inputs):
    import concourse.bass_utils as bass_utils
    from concourse.bass_utils import run_bass_kernel_spmd

    if "nc" not in _CACHE:
        _CACHE["nc"] = _build_nc()
    nc = _CACHE["nc"]

    in_maps = _prepare_core_inputs(inputs)
    trace = os.environ.get("TRNK_TRACE", "0") == "1"
    if trace:
        _install_ntff_shim()
        # no S3 in this container; keep artifacts local
        bass_utils.upload_artifacts = lambda d: d
    res = run_bass_kernel_spmd(nc, in_maps, core_ids=list(range(NCORES)),
                               trace=trace)
    _CACHE["last_results"] = res

    o_b = np.asarray(inputs["o_b"], np.float32)
    out = np.zeros((B, S, D), np.float32)
    for c in range(NCORES):
        b = c // GPC
        out[b] += res.results[c]["out"].reshape(S, D)
    out += o_b[None, None, :]
    return out
